# revision 24
# baseline (speedup 1.0000x reference)
"""Trainium2 Bass kernel for nn_AttentionBlock (sparse attention block).

Reference computation (B=4, C=512, T=2048, H=8 heads, 32 GN groups):
    xn  = GroupNorm(x) * gn_w + gn_b
    qkv = qkv_w @ xn + qkv_b            (1x1 conv)
    q,k,v = split(reshape(qkv, [B*H, 192, T])) ; each += pos
    S   = (q*s)^T (k*s),  s = ch^-0.25  => scale 1/8 on logits
    S[mask keys] = -1e9 ; P = softmax(S, axis=keys)
    h   = P @ v ; out = x + proj_w @ h + proj_b

Mask quirk (faithful to the reference): jnp.tile(mask,(H,1,1)) tiles
head-major, so attention row n = b*H + h uses mask[n % B] = mask[h % 4].

Sharding: 8 cores = (batch b, query-half j).  Each core computes
out[b][:, j*1024:(j+1)*1024] completely; host concatenates.  No collectives.

Sparsity: host compacts the key axis per mask-group m = h%4 with
keep_m = ~mask[m] (about half of T), rounded up to NK[m] = scs[m]*128.
Padded key rows get an exp-bias of -1e9 so they contribute exactly 0.

Head layout on device: slot order [0,4,1,5,2,6,3,7] so the two heads of a
mask-group (m, m+4) sit in one 128-partition pair; host reorders the qkv
weights / biases / pos / proj rows to match, so the device never permutes.

Device layout tricks: scores are computed transposed, S^T [keys, queries]:
  - the pad bias is per-partition and folds into the ACT exp for free,
  - the softmax denominator comes from an extra ones-column appended to V^T
    during the PV matmul (row 64 of the PV psum accumulates sum_s exp(S)).
GroupNorm statistics are folded on the host into a per-channel affine (A, B)
so the device applies xn = x*A + B with one tensor_scalar op per tile.

Performance structure (what made it fast):
  - ALL bf16 inputs live in ONE packed DRAM blob, split into 6 priority
    stages.  Stage n+1's dma_start is gated on stage n's completion via a
    1-element DVE probe copy that writes into stage n+1's first element
    (WAW dep) - so the stages stream strictly in priority order at full
    HBM rate instead of round-robining across queues.  Sync issues all of
    them; scalar does nothing but the exp stream (it is ~95% busy with it).
  - x_res was dropped: the residual is rebuilt on-device from the bf16 x_q
    already present (res = x_q + proj_b on DVE, during the DMA wait), and
    the output DMA is bf16 (host upconverts).  Saves ~3MB of HBM traffic
    per core.
  - a warm-up stream of ~20 junk matmuls runs during the initial DMA wait
    so the PE HAM clock-gate is at 8/8 (2.4 GHz) when real work arrives.
  - proj contracts four stacked head-pair tiles [128, T] so every proj
    pass uses the full 128-row contraction.
  - normalize is PE-free: DVE copies the denominator row to partition 0,
    GPSIMD partition_broadcast spreads it over 64 rows, a 64-lane
    reciprocal_approx_fast inverts, DVE multiplies into h_pair.
  - attention pipelines the PV matmuls behind S/exp (lead 2), per-group
    column halves, so the PE never waits on the ACT exp inside a group.
  - fp8/DoubleRow was tried and rejected: logits here reach 47 with
    softmax N_eff ~ 8, so fp8 quantization noise does not average out.
"""

import numpy as np
import ml_dtypes

B, C, T, H = 4, 512, 2048, 8
CH = C // H          # 64 channels per head
TH = T // 2          # 1024 query columns per core
P = 128
NUM_GROUPS = 32
GS = C // NUM_GROUPS  # 16 channels per group
EPS = 1e-5
BF16 = ml_dtypes.bfloat16
NMG = 4              # mask groups (= B); group m covers heads m and m+4
PERM = [0, 4, 1, 5, 2, 6, 3, 7]  # slot s holds true head PERM[s]

_graph_cache = {}


def _layout(scs):
    """Column layout of the packed bf16 input blob, plus DMA stage ranges.

    The blob streams over ONE serially-chained DMA queue in consumption
    order, cut into ~16 sub-stages so consumers wake as their data lands:
    xq blocks (-> xn affine per block), group-0 q weights, xkv0 blocks
    (-> kv affine per block), k weights, v weights + vhat init, then the
    bulk for groups 1-3 and finally the proj weights.
    """
    NK = [s * P for s in scs]
    L = {}
    off = [0]
    marks = []

    def add(name, w, mark=True):
        L[name] = (off[0], w)
        off[0] += w
        if mark:
            marks.append(off[0])

    add("xq", 4 * TH, mark=False)
    xo = L["xq"][0]
    for i in range(1, 5):
        marks.append(xo + i * TH)             # per-block sems for xn
    add("wq0", 4 * P, mark=False)
    add("posq0", TH)
    add("xkv0", 4 * NK[0], mark=False)
    ko = L["xkv0"][0]
    for i in range(1, 5):
        marks.append(ko + i * NK[0])          # per-block sems for affine
    add("poskv0", NK[0], mark=False)
    add("wk0", 4 * P)
    add("wv", 4 * 4 * P, mark=False)
    add("ident", P, mark=False)
    add("posT0", scs[0] * P)
    for m in (1, 2, 3):
        add(f"wq{m}", 4 * P, mark=False)
        add(f"wk{m}", 4 * P, mark=False)
        add(f"posq{m}", TH, mark=(m == 3))
    add("xkv1", 4 * NK[1], mark=False)
    add("poskv1", NK[1], mark=False)
    add("posT1", scs[1] * P)
    add("xkv2", 4 * NK[2], mark=False)
    add("poskv2", NK[2], mark=False)
    add("posT2", scs[2] * P)
    add("xkv3", 4 * NK[3], mark=False)
    add("poskv3", NK[3], mark=False)
    add("posT3", scs[3] * P)
    add("wp", 4 * 4 * P)
    stages = []
    lo = 0
    for hi in marks:
        stages.append((lo, hi))
        lo = hi
    return L, stages, off[0]


def _build(nkv, scs):
    """Build the Bass graph for one core (SPMD: all 8 cores run this graph)."""
    import concourse.tile as tile
    from concourse import bacc, mybir

    f32 = mybir.dt.float32
    bf16 = mybir.dt.bfloat16
    AF = mybir.ActivationFunctionType
    OP = mybir.AluOpType

    NK = [s * P for s in scs]
    L, stages, TOTAL = _layout(scs)
    # small f32 blob: pb [128,4] | pad_m [128, scs[m]] each
    pad_off = [0] * NMG
    o = 4
    for m in range(NMG):
        pad_off[m] = o
        o += scs[m]
    SW = o

    nc = bacc.Bacc("TRN2")

    d_blob = nc.dram_tensor("blob", [P, TOTAL], bf16, kind="ExternalInput")
    d_small = nc.dram_tensor("small", [P, SW], f32, kind="ExternalInput")
    d_out = nc.dram_tensor("out", [P, 2 * 4 * 512], bf16, kind="ExternalOutput")

    with tile.TileContext(nc) as tc, \
         tc.tile_pool(name="persist", bufs=1) as pers, \
         tc.tile_pool(name="mm", bufs=2, space="PSUM") as mmp, \
         tc.tile_pool(name="opool", bufs=4, space="PSUM") as opl, \
         tc.tile_pool(name="exps", bufs=6) as epl, \
         tc.tile_pool(name="nrm", bufs=1) as nrm:

        def ptile(shape, dt_, name):
            return pers.tile(shape, dt_, tag=name, name=name)

        blob = ptile([P, TOTAL], bf16, "blob")
        small = ptile([P, SW], f32, "small")

        def BL(name):
            o_, w_ = L[name]
            return blob[:, o_:o_ + w_]

        warm_in = ptile([1, 1], f32, "warm_in")
        warm_out = ptile([1, 1], f32, "warm_out")
        scr_w = ptile([P, P], bf16, "scr_w")
        scr_m = ptile([P, 512], bf16, "scr_m")

        res = ptile([P, 4 * TH], bf16, "res")
        q_sb = [ptile([P, TH], bf16, f"q{i}") for i in range(4)]
        k_sb = [ptile([P, NK[m]], bf16, f"k{m}") for m in range(NMG)]
        h_pair = [ptile([P, TH], bf16, f"h{m}") for m in range(NMG)]
        ot = [ptile([P, 4 * 512], bf16, f"ot{tb}") for tb in range(2)]
        # vhat: [v-head-a 64 | ones | v-head-b 64 | ones] per key chunk; the
        # ones columns are static (pad keys contribute exp=0 anyway).
        vhat = [ptile([P, scs[m] * 130], bf16, f"vh{m}") for m in range(NMG)]

        xkv = [BL(f"xkv{m}") for m in range(NMG)]
        posT = [BL(f"posT{m}") for m in range(NMG)]
        wq = [BL(f"wq{m}") for m in range(NMG)]
        wk = [BL(f"wk{m}") for m in range(NMG)]
        posq = [BL(f"posq{m}") for m in range(NMG)]
        poskv = [BL(f"poskv{m}") for m in range(NMG)]
        wv = BL("wv")
        wp = BL("wp")
        ident = BL("ident")
        pb = [small[:, i:i + 1] for i in range(4)]

        # ---- warm-up: memsets + junk matmul stream (PE HAM to 8/8) ----
        nc.vector.memset(warm_in, 0.0)
        nc.vector.memset(scr_w, 0.0)
        nc.vector.memset(scr_m, 0.0)
        wps = mmp.tile([P, 512], f32, tag="mm", name="warm_mm")
        for _ in range(10):
            nc.tensor.matmul(wps, scr_w, scr_m, start=True, stop=True)

        # ---- DMA: small f32 blob on scalar; the bf16 blob streams as a
        # strictly-ordered chain of sub-stage DMAs on sync.  Stage k's
        # dma_start carries a WAW dep on a 1-element probe copy that reads
        # stage k-1's last element -> the whole blob streams in priority
        # order on one queue at full rate, with a completion semaphore per
        # sub-stage so consumers wake as their slice lands.  Early gates
        # run on gpsimd (idle then); later ones on vector, emitted at
        # points where the probe's wait is already satisfied.
        nc.scalar.dma_start(small, d_small[:, :])
        nc.scalar.activation(out=warm_out, in_=warm_in, func=AF.Exp)

        lo, hi = stages[0]
        nc.sync.dma_start(blob[:, lo:hi], d_blob[:, lo:hi])

        def gate(si, eng):
            plo, phi = stages[si - 1]
            slo, shi = stages[si]
            eng.tensor_copy(out=blob[0:1, slo:slo + 1],
                            in_=blob[0:1, phi - 1:phi])
            nc.sync.dma_start(blob[:, slo:shi], d_blob[:, slo:shi])

        for si in range(1, 11):
            gate(si, nc.gpsimd)

        # GroupNorm is folded into the matmul weights on the host:
        # w_eff = w * A[batch] per input channel, and the B-part of the
        # affine becomes a bias (w @ B) folded into posq/poskv/posT.
        # The device therefore consumes x_q / x_kv RAW - no affine ops.
        xq_raw = BL("xq")

        def emit_v(m, copy_eng):
            # v^T for group m: chunk pairs share one psum tile; the pos+bias
            # term enters the psum via an identity matmul (stationary I),
            # and one strided Copy per pair moves psum -> vhat bf16.  The
            # copy runs on DVE for group 0 (ACT not yet busy... it is idle
            # but the exp stream must not queue behind copies) and on the
            # ACT engine for groups 1-3, where it fills the exp-stream gap
            # at the group boundary.
            for sp in range(0, scs[m], 2):
                w = min(2, scs[m] - sp)
                pv = mmp.tile([P, w * P], f32, tag="mm", name=f"psv{m}_{sp}")
                for u in range(w):
                    s = sp + u
                    for i in range(4):
                        nc.tensor.matmul(
                            pv[:, u * P:(u + 1) * P],
                            xkv[m][:, i * NK[m] + s * P:i * NK[m] + (s + 1) * P],
                            wv[:, m * 512 + i * P:m * 512 + (i + 1) * P],
                            start=(i == 0), stop=(i == 3))
                vh_view = vhat[m][:, sp * 130:(sp + w) * 130].rearrange(
                    "p (h c) -> p h c", c=65)[:, :, 0:CH]
                ps_view = pv.rearrange("p (h c) -> p h c", c=CH)
                pt_view = posT[m][:, sp * P:(sp + w) * P].rearrange(
                    "p (h c) -> p h c", c=CH)
                copy_eng.tensor_tensor(vh_view, ps_view, pt_view, OP.add)

        def emit_qk(m):
            pq = mmp.tile([P, TH], f32, tag="mm", name=f"psq{m}")
            for tb in range(2):
                for i in range(4):
                    nc.tensor.matmul(
                        pq[:, tb * 512:(tb + 1) * 512],
                        wq[m][:, i * P:(i + 1) * P],
                        xq_raw[:, i * TH + tb * 512:i * TH + (tb + 1) * 512],
                        start=(i == 0), stop=(i == 3))
            nc.vector.tensor_add(q_sb[m], pq, posq[m])
            nkm = NK[m]
            nb_blocks = [(st, min(512, nkm - st)) for st in range(0, nkm, 512)]
            for bi, (st, w) in enumerate(nb_blocks):
                pk = mmp.tile([P, 512], f32, tag="mm", name=f"psk{m}_{bi}")
                for i in range(4):
                    nc.tensor.matmul(
                        pk[:, 0:w],
                        wk[m][:, i * P:(i + 1) * P],
                        xkv[m][:, i * NK[m] + st:i * NK[m] + st + w],
                        start=(i == 0), stop=(i == 3))
                nc.vector.tensor_add(
                    k_sb[m][:, st:st + w], pk[:, 0:w],
                    poskv[m][:, st:st + w])

        def emit_attention(m, c0, cw, post_stage1=None):
            # pair m, query columns [c0, c0+cw): S^T -> exp -> O, with the
            # O matmuls software-pipelined one s-chunk behind S/exp so the
            # PE never waits on the ACT exp at chunk boundaries.
            nb = cw // 512
            lead = 1 if nb == 2 else 2
            o_a = [opl.tile([65, 512], f32, tag="O", name=f"oa{m}_{c0}_{t}")
                   for t in range(nb)]
            o_b = [opl.tile([65, 512], f32, tag="O", name=f"ob{m}_{c0}_{t}")
                   for t in range(nb)]
            exs = {}

            def s_stage(s):
                if nb == 1:
                    sab = mmp.tile([P, 2 * cw], f32, tag="mm",
                                   name=f"sab{m}_{c0}_{s}")
                    sv = [sab[:, 0:cw], sab[:, cw:2 * cw]]
                else:
                    sa = mmp.tile([P, cw], f32, tag="mm",
                                  name=f"sa{m}_{c0}_{s}")
                    sb_ = mmp.tile([P, cw], f32, tag="mm",
                                   name=f"sb{m}_{c0}_{s}")
                    sv = [sa, sb_]
                for t in range(nb):
                    nc.tensor.matmul(
                        sv[0][:, t * 512:(t + 1) * 512],
                        k_sb[m][0:64, s * P:(s + 1) * P],
                        q_sb[m][0:64, c0 + t * 512:c0 + (t + 1) * 512],
                        start=True, stop=True)
                for t in range(nb):
                    nc.tensor.matmul(
                        sv[1][:, t * 512:(t + 1) * 512],
                        k_sb[m][64:128, s * P:(s + 1) * P],
                        q_sb[m][64:128, c0 + t * 512:c0 + (t + 1) * 512],
                        start=True, stop=True, tile_position=(64, 0))
                ex = epl.tile([P, 2 * cw], bf16, tag="expS",
                              name=f"ex{m}_{c0}_{s}")
                pad_b = small[:, pad_off[m] + s:pad_off[m] + s + 1]
                if nb == 1:
                    # combined sab tile: ONE exp covers both heads (the pad
                    # bias is per-key, identical for both).
                    nc.scalar.activation(
                        out=ex, in_=sab, func=AF.Exp,
                        bias=pad_b, scale=0.125)
                else:
                    nc.scalar.activation(
                        out=ex[:, 0:cw], in_=sv[0], func=AF.Exp,
                        bias=pad_b, scale=0.125)
                    nc.scalar.activation(
                        out=ex[:, cw:2 * cw], in_=sv[1], func=AF.Exp,
                        bias=pad_b, scale=0.125)
                exs[s] = ex

            def o_stage(s):
                ex = exs.pop(s)
                for t in range(nb):
                    nc.tensor.matmul(
                        o_a[t], vhat[m][:, s * 130:s * 130 + 65],
                        ex[:, t * 512:(t + 1) * 512],
                        start=(s == 0), stop=(s == scs[m] - 1))
                for t in range(nb):
                    nc.tensor.matmul(
                        o_b[t], vhat[m][:, s * 130 + 65:s * 130 + 130],
                        ex[:, cw + t * 512:cw + (t + 1) * 512],
                        start=(s == 0), stop=(s == scs[m] - 1))

            for s in range(scs[m]):
                s_stage(s)
                if s == 1 and post_stage1 is not None:
                    post_stage1()
                if s >= lead:
                    o_stage(s - lead)
            for s in range(scs[m] - lead, scs[m]):
                o_stage(s)
            return o_a, o_b

        def emit_normalize(m, c0, o_a, o_b):
            # normalize: h = O[0:64] / l, l = O[64].  Entirely PE-free and
            # per-(head, col-block) so the chains pipeline: DVE copies the
            # denominator row to partition 0, GPSIMD broadcasts it over 64
            # rows, a 64-lane fast approx reciprocal inverts, DVE
            # multiplies.  (Copies run on DVE, never ACT: the scalar
            # engine is saturated with the exp stream.)
            for j, o_ in ((0, o_a), (1, o_b)):
                for t, ot_ in enumerate(o_):
                    l_sb = nrm.tile([1, 512], f32, tag=f"l{j}{t}",
                                    name=f"l{m}_{c0}_{j}{t}")
                    nc.vector.tensor_copy(out=l_sb, in_=ot_[64:65, :])
                    lb = nrm.tile([CH, 512], f32, tag=f"lb{j}{t}",
                                  name=f"lb{m}_{c0}_{j}{t}")
                    nc.gpsimd.partition_broadcast(lb, l_sb)
                    nc.vector.reciprocal_approx_fast(out=lb, in_=lb)
                    nc.vector.tensor_mul(
                        h_pair[m][j * CH:(j + 1) * CH,
                                  c0 + t * 512:c0 + (t + 1) * 512],
                        ot_[0:64, :], lb)

        # static ones columns of vhat (pad keys contribute exp=0, so an
        # all-ones column is correct for every row).  Full-tile memset; the
        # v copies then overwrite every non-ones column.
        for m in range(NMG):
            nc.vector.memset(vhat[m], 1.0)

        emit_qk(0)
        gate(11, nc.vector)         # -> groups 1-3 q/k/pos weights
        emit_v(0, nc.vector)
        gate(12, nc.vector)         # -> group 1 keys

        def mk_norm(cm, c0, oab):
            return lambda: emit_normalize(cm, c0, *oab)

        oh0 = emit_attention(0, 0, 512)
        oh1 = emit_attention(0, 512, 512, post_stage1=mk_norm(0, 0, oh0))
        carry = oh1
        gate(13, nc.vector)         # -> group 2 keys
        for m in range(1, NMG):
            # v before qk: the psum->vhat adds then land inside the
            # boundary exp-gap, gated only on the v matmuls.
            emit_v(m, nc.vector)
            emit_qk(m)
            if m == 1:
                gate(14, nc.vector)  # -> group 3 keys
            oh0 = emit_attention(
                m, 0, 512, post_stage1=mk_norm(m - 1, 512, carry))
            if m == 1:
                gate(15, nc.vector)  # -> proj weights
                # residual res = x + proj_b, from the bf16 x_q already in
                # SBUF (consumed only by the tail; emitted here in a DVE
                # lull so it never contends with norm chains).
                for i in range(4):
                    nc.vector.tensor_scalar(
                        out=res[:, i * TH:(i + 1) * TH],
                        in0=xq_raw[:, i * TH:(i + 1) * TH],
                        scalar1=pb[i], scalar2=None, op0=OP.add)
            oh1 = emit_attention(
                m, 512, 512, post_stage1=mk_norm(m, 0, oh0))
            carry = oh1

        # ---- proj + residual (contraction over 4 stacked pairs), per
        # 512-col block; output batched into one bf16 DMA per tb.
        # proj(tb0) only needs cols 0:512 of every h_pair (normalized by
        # the norm(3, 0) hook inside att(3, h1)), so it runs while the
        # last normalize chain (cols 512:) is still on DVE/GPSIMD.
        def emit_proj(tb):
            for ci in range(4):
                pp = mmp.tile([P, 512], f32, tag="mm", name=f"pp{ci}_{tb}")
                for pm in range(4):
                    nc.tensor.matmul(
                        pp, wp[:, pm * 512 + ci * P:pm * 512 + (ci + 1) * P],
                        h_pair[pm][:, tb * 512:(tb + 1) * 512],
                        start=(pm == 0), stop=(pm == 3))
                nc.vector.tensor_add(
                    ot[tb][:, ci * 512:(ci + 1) * 512], pp,
                    res[:, ci * TH + tb * 512:ci * TH + (tb + 1) * 512])
            nc.sync.dma_start(
                d_out[:, tb * 2048:(tb + 1) * 2048], ot[tb])

        emit_normalize(NMG - 1, 512, *carry)
        emit_proj(0)
        emit_proj(1)

    nc.finalize()
    return nc


def _prepare(inputs):
    """Host-side shard preparation. Returns (nkv, scs, in_maps)."""
    x = np.asarray(inputs["x"], dtype=np.float32)
    pos = np.asarray(inputs["pos"], dtype=np.float32)
    mask = np.asarray(inputs["mask"])
    gn_w = np.asarray(inputs["gn_w"], dtype=np.float32)
    gn_b = np.asarray(inputs["gn_b"], dtype=np.float32)
    qkv_w = np.asarray(inputs["qkv_w"], dtype=np.float32)
    qkv_b = np.asarray(inputs["qkv_b"], dtype=np.float32)
    proj_w = np.asarray(inputs["proj_w"], dtype=np.float32)
    proj_b = np.asarray(inputs["proj_b"], dtype=np.float32)

    # GroupNorm folded to per-channel affine per batch (stats over full T,
    # matching the reference exactly).
    xg = x.reshape(B, NUM_GROUPS, GS, T)
    mu = xg.mean(axis=(2, 3))
    var = xg.var(axis=(2, 3))
    rs = 1.0 / np.sqrt(var + EPS)
    rs_c = np.repeat(rs, GS, axis=1)
    mu_c = np.repeat(mu, GS, axis=1)
    A_all = rs_c * gn_w[None, :]
    B_all = gn_b[None, :] - mu_c * A_all

    # reorder qkv weights: reference splits rows as [h, (q|k|v), 64]; we
    # additionally permute heads into slot order PERM.
    perm = np.asarray(PERM)
    w3 = qkv_w.reshape(H, 3, CH, C)
    b3 = qkv_b.reshape(H, 3, CH)
    wq_r = w3[perm, 0].reshape(C, C)
    wk_r = w3[perm, 1].reshape(C, C)
    wv_r = w3[perm, 2].reshape(C, C)
    bq = b3[perm, 0].reshape(C)
    bk = b3[perm, 1].reshape(C)
    bv = b3[perm, 2].reshape(C)
    wqT = np.ascontiguousarray(wq_r.T)     # [in-ch, slot-out]
    wkT = np.ascontiguousarray(wk_r.T)
    wvT = np.ascontiguousarray(wv_r.T)
    # GroupNorm affine folded into the weights, per batch:
    #   w.T @ (x*A + B) = (A*w).T @ x + (w @ B)
    wqT_b = [wqT * A_all[bb][:, None] for bb in range(B)]
    wkT_b = [wkT * A_all[bb][:, None] for bb in range(B)]
    wvT_b = [wvT * A_all[bb][:, None] for bb in range(B)]
    bqB = [wq_r @ B_all[bb] for bb in range(B)]
    bkB = [wk_r @ B_all[bb] for bb in range(B)]
    bvB = [wv_r @ B_all[bb] for bb in range(B)]
    # proj: input channels permuted to slot order
    perm_idx = (perm[:, None] * CH + np.arange(CH)[None, :]).reshape(-1)
    wpT = np.ascontiguousarray(proj_w.T[perm_idx])  # [slot-in, out-ch]

    # per mask-group key compaction (mask quirk: group m uses mask[m])
    keep = [np.flatnonzero(~mask[m, 0]) for m in range(NMG)]
    scs = tuple(max((len(kp) + P - 1) // P, 1) for kp in keep)
    NK = [s * P for s in scs]
    nkv = max(NK)

    L, stages, TOTAL = _layout(scs)

    def blk4(a):
        # [512, W] -> [128, 4*W] with block i = rows [128i, 128i+128)
        w = a.shape[1]
        return a.reshape(4, P, w).transpose(1, 0, 2).reshape(P, 4 * w)

    def wblk(wT, m):
        # [128, 4*128]: block i = wT[128i:128(i+1), 128m:128(m+1)]
        return np.concatenate(
            [wT[i * P:(i + 1) * P, m * P:(m + 1) * P] for i in range(4)],
            axis=1)

    wp_piece = wpT.reshape(4, P, C).transpose(1, 0, 2).reshape(P, 4 * C)
    ident_piece = np.eye(P, dtype=np.float32)

    # per-batch xkv (shared by the two query-half cores of the batch)
    xkv_b_all = []
    for bb in range(B):
        per_m = []
        for m in range(NMG):
            kp = keep[m]
            buf = np.zeros((C, NK[m]), dtype=np.float32)
            buf[:, :len(kp)] = x[bb][:, kp]
            per_m.append(blk4(buf))
        xkv_b_all.append(per_m)

    # small f32 blob: pb 4 | pad per group
    pb_piece = proj_b.reshape(4, P).T                       # [128, 4]
    SW = 4 + sum(scs)
    sm = np.zeros((P, SW), dtype=np.float32)
    sm[:, 0:4] = pb_piece
    o = 4
    for m in range(NMG):
        padv = np.zeros(NK[m], dtype=np.float32)
        padv[len(keep[m]):] = -1e9
        sm[:, o:o + scs[m]] = padv.reshape(scs[m], P).T
        o += scs[m]

    in_maps = []
    for core in range(8):
        bb, half = core // 2, core % 2
        ts = slice(half * TH, (half + 1) * TH)
        posb = pos[bb * H:(bb + 1) * H]        # [8, 64, 2048] true head order

        pieces = {}
        pieces["xq"] = blk4(x[bb][:, ts])
        pieces["wv"] = np.concatenate(
            [wblk(wvT_b[bb], m) for m in range(NMG)], axis=1)
        pieces["wp"] = wp_piece
        pieces["ident"] = ident_piece
        posq_full = (posb[perm][:, :, ts].reshape(C, TH)
                     + (bq + bqB[bb])[:, None])
        for m in range(NMG):
            pieces[f"wq{m}"] = wblk(wqT_b[bb], m)
            pieces[f"wk{m}"] = wblk(wkT_b[bb], m)
            pieces[f"posq{m}"] = posq_full[m * P:(m + 1) * P]
            pieces[f"xkv{m}"] = xkv_b_all[bb][m]
            kp = keep[m]
            nb = len(kp)
            pkv = np.zeros((P, NK[m]), dtype=np.float32)
            posT = np.zeros((NK[m], P), dtype=np.float32)
            for j, hh in enumerate((m, m + 4)):   # slots 2m, 2m+1
                sl = slice((2 * m + j) * CH, (2 * m + j + 1) * CH)
                pkv[j * CH:(j + 1) * CH, :nb] = (
                    posb[hh][:, kp] + (bk + bkB[bb])[sl][:, None])
                posT[:nb, j * CH:(j + 1) * CH] = (
                    posb[hh][:, kp].T + (bv + bvB[bb])[sl][None, :])
            pieces[f"poskv{m}"] = pkv
            pieces[f"posT{m}"] = posT.reshape(
                scs[m], P, P).transpose(1, 0, 2).reshape(P, scs[m] * P)

        blob = np.empty((P, TOTAL), dtype=BF16)
        for name, (o_, w_) in L.items():
            assert pieces[name].shape == (P, w_), (name, pieces[name].shape, w_)
            blob[:, o_:o_ + w_] = pieces[name].astype(BF16)

        in_maps.append({"blob": blob, "small": sm})
    return nkv, scs, in_maps


def _unpack_out(raw):
    """Device out [128, 2*4*512] bf16 -> [C, TH] float32."""
    o = np.empty((C, TH), dtype=np.float32)
    for tb in range(2):
        for ci in range(4):
            o[ci * P:(ci + 1) * P, tb * 512:(tb + 1) * 512] = \
                raw[:, tb * 2048 + ci * 512:tb * 2048 + (ci + 1) * 512]
    return o


def kernel(**inputs):
    from concourse.bass_utils import run_bass_kernel_spmd

    nkv, scs, in_maps = _prepare(inputs)
    key = scs
    if key not in _graph_cache:
        _graph_cache[key] = _build(nkv, scs)
    nc = _graph_cache[key]

    res = run_bass_kernel_spmd(nc, in_maps, core_ids=list(range(8)))
    results = res.results

    out = np.empty((B, C, T), dtype=np.float32)
    for core in range(8):
        bb, half = core // 2, core % 2
        out[bb][:, half * TH:(half + 1) * TH] = _unpack_out(
            np.asarray(results[core]["out"], dtype=np.float32))
    return out


# revision 27
# speedup vs baseline: 1.3432x; 1.3432x over previous
"""Trainium2 Bass kernel for nn_AttentionBlock (sparse attention block).

Reference computation (B=4, C=512, T=2048, H=8 heads, 32 GN groups):
    xn  = GroupNorm(x) * gn_w + gn_b
    qkv = qkv_w @ xn + qkv_b            (1x1 conv)
    q,k,v = split(reshape(qkv, [B*H, 192, T])) ; each += pos
    S   = (q*s)^T (k*s),  s = ch^-0.25  => scale 1/8 on logits
    S[mask keys] = -1e9 ; P = softmax(S, axis=keys)
    h   = P @ v ; out = x + proj_w @ h + proj_b

Mask quirk (faithful to the reference): jnp.tile(mask,(H,1,1)) tiles
head-major, so attention row n = b*H + h uses mask[n % B] = mask[h % 4].

Sharding: 8 cores = (batch b, query-half j).  Each core computes
out[b][:, j*1024:(j+1)*1024] completely; host concatenates.  No collectives.

Sparsity: host compacts the key axis per mask-group m = h%4 with
keep_m = ~mask[m] (about half of T), rounded up to NK[m] = scs[m]*128.
Padded key rows get an exp-bias of -1e9 so they contribute exactly 0.

Head layout on device: slot order [0,4,1,5,2,6,3,7] so the two heads of a
mask-group (m, m+4) sit in one 128-partition pair; host reorders the qkv
weights / biases / pos / proj rows to match, so the device never permutes.

Device layout tricks: scores are computed transposed, S^T [keys, queries]:
  - the pad bias is per-partition and folds into the ACT exp for free,
  - the softmax denominator comes from an extra ones-column appended to V^T
    during the PV matmul (row 64 of the PV psum accumulates sum_s exp(S)).
GroupNorm statistics are folded on the host into a per-channel affine (A, B)
so the device applies xn = x*A + B with one tensor_scalar op per tile.

Performance structure (what made it fast):
  - ALL bf16 inputs live in ONE packed DRAM blob, split into 6 priority
    stages.  Stage n+1's dma_start is gated on stage n's completion via a
    1-element DVE probe copy that writes into stage n+1's first element
    (WAW dep) - so the stages stream strictly in priority order at full
    HBM rate instead of round-robining across queues.  Sync issues all of
    them; scalar does nothing but the exp stream (it is ~95% busy with it).
  - x_res was dropped: the residual is rebuilt on-device from the bf16 x_q
    already present (res = x_q + proj_b on DVE, during the DMA wait), and
    the output DMA is bf16 (host upconverts).  Saves ~3MB of HBM traffic
    per core.
  - a warm-up stream of ~20 junk matmuls runs during the initial DMA wait
    so the PE HAM clock-gate is at 8/8 (2.4 GHz) when real work arrives.
  - proj contracts four stacked head-pair tiles [128, T] so every proj
    pass uses the full 128-row contraction.
  - normalize is PE-free: DVE copies the denominator row to partition 0,
    GPSIMD partition_broadcast spreads it over 64 rows, a 64-lane
    reciprocal_approx_fast inverts, DVE multiplies into h_pair.
  - attention pipelines the PV matmuls behind S/exp (lead 2), per-group
    column halves, so the PE never waits on the ACT exp inside a group.
  - fp8/DoubleRow was tried and rejected: logits here reach 47 with
    softmax N_eff ~ 8, so fp8 quantization noise does not average out.
"""

import numpy as np
import ml_dtypes

B, C, T, H = 4, 512, 2048, 8
CH = C // H          # 64 channels per head
TH = T // 2          # 1024 query columns per core
P = 128
NUM_GROUPS = 32
GS = C // NUM_GROUPS  # 16 channels per group
EPS = 1e-5
BF16 = ml_dtypes.bfloat16
NMG = 4              # mask groups (= B); group m covers heads m and m+4
PERM = [0, 4, 1, 5, 2, 6, 3, 7]  # slot s holds true head PERM[s]

_graph_cache = {}


def _layout(scs):
    """Column layout of the packed bf16 input blob, plus DMA stage ranges.

    The blob streams over ONE serially-chained DMA queue in consumption
    order, cut into ~16 sub-stages so consumers wake as their data lands:
    xq blocks (-> xn affine per block), group-0 q weights, xkv0 blocks
    (-> kv affine per block), k weights, v weights + vhat init, then the
    bulk for groups 1-3 and finally the proj weights.
    """
    NK = [s * P for s in scs]
    L = {}
    off = [0]
    marks = []

    def add(name, w, mark=True):
        L[name] = (off[0], w)
        off[0] += w
        if mark:
            marks.append(off[0])

    add("xq", 4 * TH)                         # S0
    add("wq0", 4 * P, mark=False)
    add("posq0", TH, mark=False)
    add("wk0", 4 * P)                         # S1
    add("xkv0", 4 * NK[0], mark=False)
    add("poskv0", NK[0])                      # S2
    add("wv", 4 * 4 * P, mark=False)
    add("ident", P, mark=False)
    add("posT0", scs[0] * P)                  # S3
    for m in (1, 2, 3):
        add(f"wq{m}", 4 * P, mark=False)
        add(f"wk{m}", 4 * P, mark=False)
        add(f"posq{m}", TH, mark=(m == 3))    # S4
    add("xkv1", 4 * NK[1], mark=False)
    add("poskv1", NK[1], mark=False)
    add("posT1", scs[1] * P)                  # S5
    add("xkv2", 4 * NK[2], mark=False)
    add("poskv2", NK[2], mark=False)
    add("posT2", scs[2] * P)                  # S6
    add("xkv3", 4 * NK[3], mark=False)
    add("poskv3", NK[3], mark=False)
    add("posT3", scs[3] * P)                  # S7
    add("wp", 4 * 4 * P)                      # S8
    stages = []
    lo = 0
    for hi in marks:
        stages.append((lo, hi))
        lo = hi
    return L, stages, off[0]


def _build(nkv, scs):
    """Build the Bass graph for one core (SPMD: all 8 cores run this graph)."""
    import concourse.tile as tile
    from concourse import bacc, mybir

    f32 = mybir.dt.float32
    bf16 = mybir.dt.bfloat16
    AF = mybir.ActivationFunctionType
    OP = mybir.AluOpType

    NK = [s * P for s in scs]
    L, stages, TOTAL = _layout(scs)
    # small f32 blob: pb [128,4] | pad_m [128, scs[m]] each
    pad_off = [0] * NMG
    o = 4
    for m in range(NMG):
        pad_off[m] = o
        o += scs[m]
    SW = o

    nc = bacc.Bacc("TRN2")

    d_blob = nc.dram_tensor("blob", [P, TOTAL], bf16, kind="ExternalInput")
    d_small = nc.dram_tensor("small", [P, SW], f32, kind="ExternalInput")
    d_out = nc.dram_tensor("out", [P, 2 * 4 * 512], bf16, kind="ExternalOutput")

    with tile.TileContext(nc) as tc, \
         tc.tile_pool(name="persist", bufs=1) as pers, \
         tc.tile_pool(name="mm", bufs=2, space="PSUM") as mmp, \
         tc.tile_pool(name="opool", bufs=4, space="PSUM") as opl, \
         tc.tile_pool(name="exps", bufs=6) as epl, \
         tc.tile_pool(name="nrm", bufs=1) as nrm:

        def ptile(shape, dt_, name):
            return pers.tile(shape, dt_, tag=name, name=name)

        blob = ptile([P, TOTAL], bf16, "blob")
        small = ptile([P, SW], f32, "small")

        def BL(name):
            o_, w_ = L[name]
            return blob[:, o_:o_ + w_]

        warm_in = ptile([1, 1], f32, "warm_in")
        warm_out = ptile([1, 1], f32, "warm_out")
        scr_w = ptile([P, P], bf16, "scr_w")
        scr_m = ptile([P, 512], bf16, "scr_m")

        res = ptile([P, 4 * TH], bf16, "res")
        q_sb = [ptile([P, TH], bf16, f"q{i}") for i in range(4)]
        k_sb = [ptile([P, NK[m]], bf16, f"k{m}") for m in range(NMG)]
        h_pair = [ptile([P, TH], bf16, f"h{m}") for m in range(NMG)]
        ot = [ptile([P, 4 * 512], bf16, f"ot{tb}") for tb in range(2)]
        # vhat: [v-head-a 64 | ones | v-head-b 64 | ones] per key chunk; the
        # ones columns are static (pad keys contribute exp=0 anyway).
        vhat = [ptile([P, scs[m] * 130], bf16, f"vh{m}") for m in range(NMG)]

        xkv = [BL(f"xkv{m}") for m in range(NMG)]
        posT = [BL(f"posT{m}") for m in range(NMG)]
        wq = [BL(f"wq{m}") for m in range(NMG)]
        wk = [BL(f"wk{m}") for m in range(NMG)]
        posq = [BL(f"posq{m}") for m in range(NMG)]
        poskv = [BL(f"poskv{m}") for m in range(NMG)]
        wv = BL("wv")
        wp = BL("wp")
        ident = BL("ident")
        pb = [small[:, i:i + 1] for i in range(4)]

        # ---- warm-up: memsets + junk matmul stream (PE HAM to 8/8) ----
        nc.vector.memset(warm_in, 0.0)
        nc.vector.memset(scr_w, 0.0)
        nc.vector.memset(scr_m, 0.0)
        wps = mmp.tile([P, 512], f32, tag="mm", name="warm_mm")
        for _ in range(10):
            nc.tensor.matmul(wps, scr_w, scr_m, start=True, stop=True)

        # ---- DMA: small f32 blob on scalar; the bf16 blob streams as a
        # priority-ordered chain of stage DMAs on sync, pipelined two
        # deep: stage k's dma_start carries a WAW dep on a 1-element probe
        # copy that reads stage k-2's last element.  Two stages stream
        # concurrently (so the ~2us per-DMA completion latency is hidden)
        # while later stages cannot steal bandwidth from earlier ones.
        nc.scalar.dma_start(small, d_small[:, :])
        nc.scalar.activation(out=warm_out, in_=warm_in, func=AF.Exp)

        for si in (0, 1):
            lo, hi = stages[si]
            nc.sync.dma_start(blob[:, lo:hi], d_blob[:, lo:hi])

        def gate(si, eng):
            plo, phi = stages[si - 2]
            slo, shi = stages[si]
            eng.tensor_copy(out=blob[0:1, slo:slo + 1],
                            in_=blob[0:1, phi - 1:phi])
            nc.sync.dma_start(blob[:, slo:shi], d_blob[:, slo:shi])

        for si in range(2, 8):
            gate(si, nc.gpsimd)

        # GroupNorm is folded into the matmul weights on the host:
        # w_eff = w * A[batch] per input channel, and the B-part of the
        # affine becomes a bias (w @ B) folded into posq/poskv/posT.
        # The device therefore consumes x_q / x_kv RAW - no affine ops.
        xq_raw = BL("xq")

        def emit_v(m, copy_eng):
            # v^T for group m: chunk pairs share one psum tile; the pos+bias
            # term enters the psum via an identity matmul (stationary I),
            # and one strided Copy per pair moves psum -> vhat bf16.  The
            # copy runs on DVE for group 0 (ACT not yet busy... it is idle
            # but the exp stream must not queue behind copies) and on the
            # ACT engine for groups 1-3, where it fills the exp-stream gap
            # at the group boundary.
            for sp in range(0, scs[m], 2):
                w = min(2, scs[m] - sp)
                pv = mmp.tile([P, w * P], f32, tag="mm", name=f"psv{m}_{sp}")
                for u in range(w):
                    s = sp + u
                    for i in range(4):
                        nc.tensor.matmul(
                            pv[:, u * P:(u + 1) * P],
                            xkv[m][:, i * NK[m] + s * P:i * NK[m] + (s + 1) * P],
                            wv[:, m * 512 + i * P:m * 512 + (i + 1) * P],
                            start=(i == 0), stop=(i == 3))
                vh_view = vhat[m][:, sp * 130:(sp + w) * 130].rearrange(
                    "p (h c) -> p h c", c=65)[:, :, 0:CH]
                ps_view = pv.rearrange("p (h c) -> p h c", c=CH)
                pt_view = posT[m][:, sp * P:(sp + w) * P].rearrange(
                    "p (h c) -> p h c", c=CH)
                copy_eng.tensor_tensor(vh_view, ps_view, pt_view, OP.add)

        def emit_qk(m):
            pq = mmp.tile([P, TH], f32, tag="mm", name=f"psq{m}")
            for tb in range(2):
                for i in range(4):
                    nc.tensor.matmul(
                        pq[:, tb * 512:(tb + 1) * 512],
                        wq[m][:, i * P:(i + 1) * P],
                        xq_raw[:, i * TH + tb * 512:i * TH + (tb + 1) * 512],
                        start=(i == 0), stop=(i == 3))
            nc.vector.tensor_add(q_sb[m], pq, posq[m])
            nkm = NK[m]
            nb_blocks = [(st, min(512, nkm - st)) for st in range(0, nkm, 512)]
            for bi, (st, w) in enumerate(nb_blocks):
                pk = mmp.tile([P, 512], f32, tag="mm", name=f"psk{m}_{bi}")
                for i in range(4):
                    nc.tensor.matmul(
                        pk[:, 0:w],
                        wk[m][:, i * P:(i + 1) * P],
                        xkv[m][:, i * NK[m] + st:i * NK[m] + st + w],
                        start=(i == 0), stop=(i == 3))
                nc.vector.tensor_add(
                    k_sb[m][:, st:st + w], pk[:, 0:w],
                    poskv[m][:, st:st + w])

        def emit_attention(m, c0, cw, post_stage1=None):
            # pair m, query columns [c0, c0+cw): S^T -> exp -> O, with the
            # O matmuls software-pipelined one s-chunk behind S/exp so the
            # PE never waits on the ACT exp at chunk boundaries.
            nb = cw // 512
            lead = 1 if nb == 2 else 2
            o_a = [opl.tile([65, 512], f32, tag="O", name=f"oa{m}_{c0}_{t}")
                   for t in range(nb)]
            o_b = [opl.tile([65, 512], f32, tag="O", name=f"ob{m}_{c0}_{t}")
                   for t in range(nb)]
            exs = {}

            def s_stage(s):
                if nb == 1:
                    sab = mmp.tile([P, 2 * cw], f32, tag="mm",
                                   name=f"sab{m}_{c0}_{s}")
                    sv = [sab[:, 0:cw], sab[:, cw:2 * cw]]
                else:
                    sa = mmp.tile([P, cw], f32, tag="mm",
                                  name=f"sa{m}_{c0}_{s}")
                    sb_ = mmp.tile([P, cw], f32, tag="mm",
                                   name=f"sb{m}_{c0}_{s}")
                    sv = [sa, sb_]
                for t in range(nb):
                    nc.tensor.matmul(
                        sv[0][:, t * 512:(t + 1) * 512],
                        k_sb[m][0:64, s * P:(s + 1) * P],
                        q_sb[m][0:64, c0 + t * 512:c0 + (t + 1) * 512],
                        start=True, stop=True)
                for t in range(nb):
                    nc.tensor.matmul(
                        sv[1][:, t * 512:(t + 1) * 512],
                        k_sb[m][64:128, s * P:(s + 1) * P],
                        q_sb[m][64:128, c0 + t * 512:c0 + (t + 1) * 512],
                        start=True, stop=True, tile_position=(64, 0))
                ex = epl.tile([P, 2 * cw], bf16, tag="expS",
                              name=f"ex{m}_{c0}_{s}")
                pad_b = small[:, pad_off[m] + s:pad_off[m] + s + 1]
                if nb == 1:
                    # combined sab tile: ONE exp covers both heads (the pad
                    # bias is per-key, identical for both).
                    nc.scalar.activation(
                        out=ex, in_=sab, func=AF.Exp,
                        bias=pad_b, scale=0.125)
                else:
                    nc.scalar.activation(
                        out=ex[:, 0:cw], in_=sv[0], func=AF.Exp,
                        bias=pad_b, scale=0.125)
                    nc.scalar.activation(
                        out=ex[:, cw:2 * cw], in_=sv[1], func=AF.Exp,
                        bias=pad_b, scale=0.125)
                exs[s] = ex

            def o_stage(s):
                ex = exs.pop(s)
                for t in range(nb):
                    nc.tensor.matmul(
                        o_a[t], vhat[m][:, s * 130:s * 130 + 65],
                        ex[:, t * 512:(t + 1) * 512],
                        start=(s == 0), stop=(s == scs[m] - 1))
                for t in range(nb):
                    nc.tensor.matmul(
                        o_b[t], vhat[m][:, s * 130 + 65:s * 130 + 130],
                        ex[:, cw + t * 512:cw + (t + 1) * 512],
                        start=(s == 0), stop=(s == scs[m] - 1))

            for s in range(scs[m]):
                s_stage(s)
                if s == 1 and post_stage1 is not None:
                    post_stage1()
                if s >= lead:
                    o_stage(s - lead)
            for s in range(scs[m] - lead, scs[m]):
                o_stage(s)
            return o_a, o_b

        def emit_normalize(m, c0, o_a, o_b):
            # normalize: h = O[0:64] / l, l = O[64].  Entirely PE-free and
            # per-(head, col-block) so the chains pipeline: DVE copies the
            # denominator row to partition 0, GPSIMD broadcasts it over 64
            # rows, a 64-lane fast approx reciprocal inverts, DVE
            # multiplies.  (Copies run on DVE, never ACT: the scalar
            # engine is saturated with the exp stream.)
            for j, o_ in ((0, o_a), (1, o_b)):
                for t, ot_ in enumerate(o_):
                    l_sb = nrm.tile([1, 512], f32, tag=f"l{j}{t}",
                                    name=f"l{m}_{c0}_{j}{t}")
                    nc.vector.tensor_copy(out=l_sb, in_=ot_[64:65, :])
                    lb = nrm.tile([CH, 512], f32, tag=f"lb{j}{t}",
                                  name=f"lb{m}_{c0}_{j}{t}")
                    nc.gpsimd.partition_broadcast(lb, l_sb)
                    nc.vector.reciprocal_approx_fast(out=lb, in_=lb)
                    nc.vector.tensor_mul(
                        h_pair[m][j * CH:(j + 1) * CH,
                                  c0 + t * 512:c0 + (t + 1) * 512],
                        ot_[0:64, :], lb)

        # static ones columns of vhat (pad keys contribute exp=0, so an
        # all-ones column is correct for every row).  Full-tile memset; the
        # v copies then overwrite every non-ones column.
        for m in range(NMG):
            nc.vector.memset(vhat[m], 1.0)

        emit_qk(0)
        emit_v(0, nc.vector)

        def mk_norm(cm, c0, oab):
            return lambda: emit_normalize(cm, c0, *oab)

        oh0 = emit_attention(0, 0, 512)
        oh1 = emit_attention(0, 512, 512, post_stage1=mk_norm(0, 0, oh0))
        carry = oh1
        gate(8, nc.vector)          # -> proj weights
        for m in range(1, NMG):
            # v before qk: the psum->vhat adds then land inside the
            # boundary exp-gap, gated only on the v matmuls.
            emit_v(m, nc.vector)
            emit_qk(m)
            oh0 = emit_attention(
                m, 0, 512, post_stage1=mk_norm(m - 1, 512, carry))
            if m == 1:
                # residual res = x + proj_b, from the bf16 x_q already in
                # SBUF (consumed only by the tail; emitted here in a DVE
                # lull so it never contends with norm chains).
                for i in range(4):
                    nc.vector.tensor_scalar(
                        out=res[:, i * TH:(i + 1) * TH],
                        in0=xq_raw[:, i * TH:(i + 1) * TH],
                        scalar1=pb[i], scalar2=None, op0=OP.add)
            oh1 = emit_attention(
                m, 512, 512, post_stage1=mk_norm(m, 0, oh0))
            carry = oh1

        # ---- proj + residual (contraction over 4 stacked pairs), per
        # 512-col block; output batched into one bf16 DMA per tb.
        # proj(tb0) only needs cols 0:512 of every h_pair (normalized by
        # the norm(3, 0) hook inside att(3, h1)), so it runs while the
        # last normalize chain (cols 512:) is still on DVE/GPSIMD.
        def emit_proj(tb):
            for ci in range(4):
                pp = mmp.tile([P, 512], f32, tag="mm", name=f"pp{ci}_{tb}")
                for pm in range(4):
                    nc.tensor.matmul(
                        pp, wp[:, pm * 512 + ci * P:pm * 512 + (ci + 1) * P],
                        h_pair[pm][:, tb * 512:(tb + 1) * 512],
                        start=(pm == 0), stop=(pm == 3))
                nc.vector.tensor_add(
                    ot[tb][:, ci * 512:(ci + 1) * 512], pp,
                    res[:, ci * TH + tb * 512:ci * TH + (tb + 1) * 512])
            nc.sync.dma_start(
                d_out[:, tb * 2048:(tb + 1) * 2048], ot[tb])

        emit_normalize(NMG - 1, 512, *carry)
        emit_proj(0)
        emit_proj(1)

    nc.finalize()
    return nc


def _prepare(inputs):
    """Host-side shard preparation. Returns (nkv, scs, in_maps)."""
    x = np.asarray(inputs["x"], dtype=np.float32)
    pos = np.asarray(inputs["pos"], dtype=np.float32)
    mask = np.asarray(inputs["mask"])
    gn_w = np.asarray(inputs["gn_w"], dtype=np.float32)
    gn_b = np.asarray(inputs["gn_b"], dtype=np.float32)
    qkv_w = np.asarray(inputs["qkv_w"], dtype=np.float32)
    qkv_b = np.asarray(inputs["qkv_b"], dtype=np.float32)
    proj_w = np.asarray(inputs["proj_w"], dtype=np.float32)
    proj_b = np.asarray(inputs["proj_b"], dtype=np.float32)

    # GroupNorm folded to per-channel affine per batch (stats over full T,
    # matching the reference exactly).
    xg = x.reshape(B, NUM_GROUPS, GS, T)
    mu = xg.mean(axis=(2, 3))
    var = xg.var(axis=(2, 3))
    rs = 1.0 / np.sqrt(var + EPS)
    rs_c = np.repeat(rs, GS, axis=1)
    mu_c = np.repeat(mu, GS, axis=1)
    A_all = rs_c * gn_w[None, :]
    B_all = gn_b[None, :] - mu_c * A_all

    # reorder qkv weights: reference splits rows as [h, (q|k|v), 64]; we
    # additionally permute heads into slot order PERM.
    perm = np.asarray(PERM)
    w3 = qkv_w.reshape(H, 3, CH, C)
    b3 = qkv_b.reshape(H, 3, CH)
    wq_r = w3[perm, 0].reshape(C, C)
    wk_r = w3[perm, 1].reshape(C, C)
    wv_r = w3[perm, 2].reshape(C, C)
    bq = b3[perm, 0].reshape(C)
    bk = b3[perm, 1].reshape(C)
    bv = b3[perm, 2].reshape(C)
    wqT = np.ascontiguousarray(wq_r.T)     # [in-ch, slot-out]
    wkT = np.ascontiguousarray(wk_r.T)
    wvT = np.ascontiguousarray(wv_r.T)
    # GroupNorm affine folded into the weights, per batch:
    #   w.T @ (x*A + B) = (A*w).T @ x + (w @ B)
    wqT_b = [wqT * A_all[bb][:, None] for bb in range(B)]
    wkT_b = [wkT * A_all[bb][:, None] for bb in range(B)]
    wvT_b = [wvT * A_all[bb][:, None] for bb in range(B)]
    bqB = [wq_r @ B_all[bb] for bb in range(B)]
    bkB = [wk_r @ B_all[bb] for bb in range(B)]
    bvB = [wv_r @ B_all[bb] for bb in range(B)]
    # proj: input channels permuted to slot order
    perm_idx = (perm[:, None] * CH + np.arange(CH)[None, :]).reshape(-1)
    wpT = np.ascontiguousarray(proj_w.T[perm_idx])  # [slot-in, out-ch]

    # per mask-group key compaction (mask quirk: group m uses mask[m])
    keep = [np.flatnonzero(~mask[m, 0]) for m in range(NMG)]
    scs = tuple(max((len(kp) + P - 1) // P, 1) for kp in keep)
    NK = [s * P for s in scs]
    nkv = max(NK)

    L, stages, TOTAL = _layout(scs)

    def blk4(a):
        # [512, W] -> [128, 4*W] with block i = rows [128i, 128i+128)
        w = a.shape[1]
        return a.reshape(4, P, w).transpose(1, 0, 2).reshape(P, 4 * w)

    def wblk(wT, m):
        # [128, 4*128]: block i = wT[128i:128(i+1), 128m:128(m+1)]
        return np.concatenate(
            [wT[i * P:(i + 1) * P, m * P:(m + 1) * P] for i in range(4)],
            axis=1)

    wp_piece = wpT.reshape(4, P, C).transpose(1, 0, 2).reshape(P, 4 * C)
    ident_piece = np.eye(P, dtype=np.float32)

    # per-batch xkv (shared by the two query-half cores of the batch)
    xkv_b_all = []
    for bb in range(B):
        per_m = []
        for m in range(NMG):
            kp = keep[m]
            buf = np.zeros((C, NK[m]), dtype=np.float32)
            buf[:, :len(kp)] = x[bb][:, kp]
            per_m.append(blk4(buf))
        xkv_b_all.append(per_m)

    # small f32 blob: pb 4 | pad per group
    pb_piece = proj_b.reshape(4, P).T                       # [128, 4]
    SW = 4 + sum(scs)
    sm = np.zeros((P, SW), dtype=np.float32)
    sm[:, 0:4] = pb_piece
    o = 4
    for m in range(NMG):
        padv = np.zeros(NK[m], dtype=np.float32)
        padv[len(keep[m]):] = -1e9
        sm[:, o:o + scs[m]] = padv.reshape(scs[m], P).T
        o += scs[m]

    in_maps = []
    for core in range(8):
        bb, half = core // 2, core % 2
        ts = slice(half * TH, (half + 1) * TH)
        posb = pos[bb * H:(bb + 1) * H]        # [8, 64, 2048] true head order

        pieces = {}
        pieces["xq"] = blk4(x[bb][:, ts])
        pieces["wv"] = np.concatenate(
            [wblk(wvT_b[bb], m) for m in range(NMG)], axis=1)
        pieces["wp"] = wp_piece
        pieces["ident"] = ident_piece
        posq_full = (posb[perm][:, :, ts].reshape(C, TH)
                     + (bq + bqB[bb])[:, None])
        for m in range(NMG):
            pieces[f"wq{m}"] = wblk(wqT_b[bb], m)
            pieces[f"wk{m}"] = wblk(wkT_b[bb], m)
            pieces[f"posq{m}"] = posq_full[m * P:(m + 1) * P]
            pieces[f"xkv{m}"] = xkv_b_all[bb][m]
            kp = keep[m]
            nb = len(kp)
            pkv = np.zeros((P, NK[m]), dtype=np.float32)
            posT = np.zeros((NK[m], P), dtype=np.float32)
            for j, hh in enumerate((m, m + 4)):   # slots 2m, 2m+1
                sl = slice((2 * m + j) * CH, (2 * m + j + 1) * CH)
                pkv[j * CH:(j + 1) * CH, :nb] = (
                    posb[hh][:, kp] + (bk + bkB[bb])[sl][:, None])
                posT[:nb, j * CH:(j + 1) * CH] = (
                    posb[hh][:, kp].T + (bv + bvB[bb])[sl][None, :])
            pieces[f"poskv{m}"] = pkv
            pieces[f"posT{m}"] = posT.reshape(
                scs[m], P, P).transpose(1, 0, 2).reshape(P, scs[m] * P)

        blob = np.empty((P, TOTAL), dtype=BF16)
        for name, (o_, w_) in L.items():
            assert pieces[name].shape == (P, w_), (name, pieces[name].shape, w_)
            blob[:, o_:o_ + w_] = pieces[name].astype(BF16)

        in_maps.append({"blob": blob, "small": sm})
    return nkv, scs, in_maps


def _unpack_out(raw):
    """Device out [128, 2*4*512] bf16 -> [C, TH] float32."""
    o = np.empty((C, TH), dtype=np.float32)
    for tb in range(2):
        for ci in range(4):
            o[ci * P:(ci + 1) * P, tb * 512:(tb + 1) * 512] = \
                raw[:, tb * 2048 + ci * 512:tb * 2048 + (ci + 1) * 512]
    return o


def kernel(**inputs):
    from concourse.bass_utils import run_bass_kernel_spmd

    nkv, scs, in_maps = _prepare(inputs)
    key = scs
    if key not in _graph_cache:
        _graph_cache[key] = _build(nkv, scs)
    nc = _graph_cache[key]

    res = run_bass_kernel_spmd(nc, in_maps, core_ids=list(range(8)))
    results = res.results

    out = np.empty((B, C, T), dtype=np.float32)
    for core in range(8):
        bb, half = core // 2, core % 2
        out[bb][:, half * TH:(half + 1) * TH] = _unpack_out(
            np.asarray(results[core]["out"], dtype=np.float32))
    return out


# revision 34
# speedup vs baseline: 1.3656x; 1.0167x over previous
"""Trainium2 Bass kernel for nn_AttentionBlock (sparse attention block).

Reference computation (B=4, C=512, T=2048, H=8 heads, 32 GN groups):
    xn  = GroupNorm(x) * gn_w + gn_b
    qkv = qkv_w @ xn + qkv_b            (1x1 conv)
    q,k,v = split(reshape(qkv, [B*H, 192, T])) ; each += pos
    S   = (q*s)^T (k*s),  s = ch^-0.25  => scale 1/8 on logits
    S[mask keys] = -1e9 ; P = softmax(S, axis=keys)
    h   = P @ v ; out = x + proj_w @ h + proj_b

Mask quirk (faithful to the reference): jnp.tile(mask,(H,1,1)) tiles
head-major, so attention row n = b*H + h uses mask[n % B] = mask[h % 4].

Sharding: 8 cores = (batch b, query-half j).  Each core computes
out[b][:, j*1024:(j+1)*1024] completely; host concatenates.  No collectives.

Sparsity: host compacts the key axis per mask-group m = h%4 with
keep_m = ~mask[m] (about half of T), rounded up to NK[m] = scs[m]*128.
Padded key rows get an exp-bias of -1e9 so they contribute exactly 0.

Head layout on device: slot order [0,4,1,5,2,6,3,7] so the two heads of a
mask-group (m, m+4) sit in one 128-partition pair; host reorders the qkv
weights / biases / pos / proj rows to match, so the device never permutes.

Device layout tricks: scores are computed transposed, S^T [keys, queries]:
  - the pad bias is per-partition and folds into the ACT exp for free,
  - the softmax denominator comes from an extra ones-column appended to V^T
    during the PV matmul (row 64 of the PV psum accumulates sum_s exp(S)).
GroupNorm statistics are folded on the host into a per-channel affine (A, B)
so the device applies xn = x*A + B with one tensor_scalar op per tile.

Performance structure (what made it fast):
  - ALL bf16 inputs live in ONE packed DRAM blob, split into 6 priority
    stages.  Stage n+1's dma_start is gated on stage n's completion via a
    1-element DVE probe copy that writes into stage n+1's first element
    (WAW dep) - so the stages stream strictly in priority order at full
    HBM rate instead of round-robining across queues.  Sync issues all of
    them; scalar does nothing but the exp stream (it is ~95% busy with it).
  - x_res was dropped: the residual is rebuilt on-device from the bf16 x_q
    already present (res = x_q + proj_b on DVE, during the DMA wait), and
    the output DMA is bf16 (host upconverts).  Saves ~3MB of HBM traffic
    per core.
  - a warm-up stream of ~20 junk matmuls runs during the initial DMA wait
    so the PE HAM clock-gate is at 8/8 (2.4 GHz) when real work arrives.
  - proj contracts four stacked head-pair tiles [128, T] so every proj
    pass uses the full 128-row contraction.
  - normalize is PE-free: DVE copies the denominator row to partition 0,
    GPSIMD partition_broadcast spreads it over 64 rows, a 64-lane
    reciprocal_approx_fast inverts, DVE multiplies into h_pair.
  - attention pipelines the PV matmuls behind S/exp (lead 2), per-group
    column halves, so the PE never waits on the ACT exp inside a group.
  - fp8/DoubleRow was tried and rejected: logits here reach 47 with
    softmax N_eff ~ 8, so fp8 quantization noise does not average out.
"""

import numpy as np
import ml_dtypes

B, C, T, H = 4, 512, 2048, 8
CH = C // H          # 64 channels per head
TH = T // 2          # 1024 query columns per core
P = 128
NUM_GROUPS = 32
GS = C // NUM_GROUPS  # 16 channels per group
EPS = 1e-5
BF16 = ml_dtypes.bfloat16
NMG = 4              # mask groups (= B); group m covers heads m and m+4
PERM = [0, 4, 1, 5, 2, 6, 3, 7]  # slot s holds true head PERM[s]

_graph_cache = {}


def _layout(scs):
    """Column layout of the packed bf16 input blob, plus DMA stage ranges.

    The blob streams over ONE serially-chained DMA queue in consumption
    order, cut into ~16 sub-stages so consumers wake as their data lands:
    xq blocks (-> xn affine per block), group-0 q weights, xkv0 blocks
    (-> kv affine per block), k weights, v weights + vhat init, then the
    bulk for groups 1-3 and finally the proj weights.
    """
    NK = [s * P for s in scs]
    L = {}
    off = [0]
    marks = []

    def add(name, w, mark=True):
        L[name] = (off[0], w)
        off[0] += w
        if mark:
            marks.append(off[0])

    add("xq", 4 * TH)                         # S0
    add("wq0", 4 * P, mark=False)
    add("posq0", TH, mark=False)
    add("wk0", 4 * P, mark=False)
    add("wv", 4 * 4 * P, mark=False)
    add("ident", P)                           # S1
    add("xkv0", 4 * NK[0], mark=False)
    add("poskv0", NK[0])                      # S2
    add("posT0", scs[0] * P)                  # S3
    for m in (1, 2, 3):
        add(f"wq{m}", 4 * P, mark=False)
        add(f"wk{m}", 4 * P, mark=False)
        add(f"posq{m}", TH, mark=(m == 3))    # S4
    add("xkv1", 4 * NK[1], mark=False)
    add("poskv1", NK[1], mark=False)
    add("posT1", scs[1] * P)                  # S5
    add("xkv2", 4 * NK[2], mark=False)
    add("poskv2", NK[2], mark=False)
    add("posT2", scs[2] * P)                  # S6
    add("xkv3", 4 * NK[3], mark=False)
    add("poskv3", NK[3], mark=False)
    add("posT3", scs[3] * P)                  # S7
    add("wp", 4 * 4 * P)                      # S8
    stages = []
    lo = 0
    for hi in marks:
        stages.append((lo, hi))
        lo = hi
    return L, stages, off[0]


def _build(nkv, scs):
    """Build the Bass graph for one core (SPMD: all 8 cores run this graph)."""
    import concourse.tile as tile
    from concourse import bacc, mybir

    f32 = mybir.dt.float32
    bf16 = mybir.dt.bfloat16
    AF = mybir.ActivationFunctionType
    OP = mybir.AluOpType

    NK = [s * P for s in scs]
    L, stages, TOTAL = _layout(scs)
    # small f32 blob: pb [128,4] | pad_m [128, scs[m]] each
    pad_off = [0] * NMG
    o = 4
    for m in range(NMG):
        pad_off[m] = o
        o += scs[m]
    SW = o

    nc = bacc.Bacc("TRN2")

    d_blob = nc.dram_tensor("blob", [P, TOTAL], bf16, kind="ExternalInput")
    d_small = nc.dram_tensor("small", [P, SW], f32, kind="ExternalInput")
    d_out = nc.dram_tensor("out", [P, 2 * 4 * 512], bf16, kind="ExternalOutput")

    with tile.TileContext(nc) as tc, \
         tc.tile_pool(name="persist", bufs=1) as pers, \
         tc.tile_pool(name="mm", bufs=2, space="PSUM") as mmp, \
         tc.tile_pool(name="opool", bufs=4, space="PSUM") as opl, \
         tc.tile_pool(name="exps", bufs=6) as epl, \
         tc.tile_pool(name="nrm", bufs=1) as nrm:

        def ptile(shape, dt_, name):
            return pers.tile(shape, dt_, tag=name, name=name)

        blob = ptile([P, TOTAL], bf16, "blob")
        small = ptile([P, SW], f32, "small")

        def BL(name):
            o_, w_ = L[name]
            return blob[:, o_:o_ + w_]

        warm_in = ptile([1, 1], f32, "warm_in")
        warm_out = ptile([1, 1], f32, "warm_out")
        scr_w = ptile([P, P], bf16, "scr_w")
        scr_m = ptile([P, 512], bf16, "scr_m")

        res = ptile([P, 4 * TH], bf16, "res")
        q_sb = [ptile([P, TH], bf16, f"q{i}") for i in range(4)]
        k_sb = [ptile([P, NK[m]], bf16, f"k{m}") for m in range(NMG)]
        h_pair = [ptile([P, TH], bf16, f"h{m}") for m in range(NMG)]
        ot = [ptile([P, 4 * 512], bf16, f"ot{tb}") for tb in range(2)]
        # vhat: [v-head-a 64 | ones | v-head-b 64 | ones] per key chunk; the
        # ones columns are static (pad keys contribute exp=0 anyway).
        vhat = [ptile([P, scs[m] * 130], bf16, f"vh{m}") for m in range(NMG)]

        xkv = [BL(f"xkv{m}") for m in range(NMG)]
        posT = [BL(f"posT{m}") for m in range(NMG)]
        wq = [BL(f"wq{m}") for m in range(NMG)]
        wk = [BL(f"wk{m}") for m in range(NMG)]
        posq = [BL(f"posq{m}") for m in range(NMG)]
        poskv = [BL(f"poskv{m}") for m in range(NMG)]
        wv = BL("wv")
        wp = BL("wp")
        ident = BL("ident")
        pb = [small[:, i:i + 1] for i in range(4)]

        # ---- warm-up: memsets + junk matmul stream (PE HAM to 8/8) ----
        nc.vector.memset(warm_in, 0.0)
        nc.vector.memset(scr_w, 0.0)
        nc.vector.memset(scr_m, 0.0)
        wps = mmp.tile([P, 512], f32, tag="mm", name="warm_mm")
        for _ in range(14):
            nc.tensor.matmul(wps, scr_w, scr_m, start=True, stop=True)

        # ---- DMA: small f32 blob on scalar; the bf16 blob streams as a
        # priority-ordered chain of stage DMAs on sync, pipelined two
        # deep: stage k's dma_start carries a WAW dep on a 1-element probe
        # copy that reads stage k-2's last element.  Two stages stream
        # concurrently (so the ~2us per-DMA completion latency is hidden)
        # while later stages cannot steal bandwidth from earlier ones.
        nc.scalar.dma_start(small, d_small[:, :])
        nc.scalar.activation(out=warm_out, in_=warm_in, func=AF.Exp)

        for si in (0, 1):
            lo, hi = stages[si]
            nc.sync.dma_start(blob[:, lo:hi], d_blob[:, lo:hi])

        def gate(si, eng):
            plo, phi = stages[si - 2]
            slo, shi = stages[si]
            eng.tensor_copy(out=blob[0:1, slo:slo + 1],
                            in_=blob[0:1, phi - 1:phi])
            nc.sync.dma_start(blob[:, slo:shi], d_blob[:, slo:shi])

        for si in range(2, 8):
            gate(si, nc.gpsimd)

        # GroupNorm is folded into the matmul weights on the host:
        # w_eff = w * A[batch] per input channel, and the B-part of the
        # affine becomes a bias (w @ B) folded into posq/poskv/posT.
        # The device therefore consumes x_q / x_kv RAW - no affine ops.
        xq_raw = BL("xq")

        def v_jobs(m):
            # v^T for group m, as a list of per-chunk-pair closures that
            # emit_attention interleaves into its s-loop (one pair every
            # two chunks, in the exp-paced PE slack): 8+ matmuls into a
            # psum pair tile, then one DVE add (psum + posT -> vhat bf16).
            def mk(sp):
                def job():
                    w = min(2, scs[m] - sp)
                    pv = mmp.tile([P, w * P], f32, tag="mm",
                                  name=f"psv{m}_{sp}")
                    for u in range(w):
                        s = sp + u
                        for i in range(4):
                            nc.tensor.matmul(
                                pv[:, u * P:(u + 1) * P],
                                xkv[m][:, i * NK[m] + s * P:
                                       i * NK[m] + (s + 1) * P],
                                wv[:, m * 512 + i * P:m * 512 + (i + 1) * P],
                                start=(i == 0), stop=(i == 3))
                    vh_view = vhat[m][:, sp * 130:(sp + w) * 130].rearrange(
                        "p (h c) -> p h c", c=65)[:, :, 0:CH]
                    ps_view = pv.rearrange("p (h c) -> p h c", c=CH)
                    pt_view = posT[m][:, sp * P:(sp + w) * P].rearrange(
                        "p (h c) -> p h c", c=CH)
                    nc.vector.tensor_tensor(vh_view, ps_view, pt_view, OP.add)
                return job
            return [mk(sp) for sp in range(0, scs[m], 2)]

        def emit_qk(m):
            pq = mmp.tile([P, TH], f32, tag="mm", name=f"psq{m}")
            for tb in range(2):
                for i in range(4):
                    nc.tensor.matmul(
                        pq[:, tb * 512:(tb + 1) * 512],
                        wq[m][:, i * P:(i + 1) * P],
                        xq_raw[:, i * TH + tb * 512:i * TH + (tb + 1) * 512],
                        start=(i == 0), stop=(i == 3))
            nc.vector.tensor_add(q_sb[m], pq, posq[m])
            nkm = NK[m]
            nb_blocks = [(st, min(512, nkm - st)) for st in range(0, nkm, 512)]
            for bi, (st, w) in enumerate(nb_blocks):
                pk = mmp.tile([P, 512], f32, tag="mm", name=f"psk{m}_{bi}")
                for i in range(4):
                    nc.tensor.matmul(
                        pk[:, 0:w],
                        wk[m][:, i * P:(i + 1) * P],
                        xkv[m][:, i * NK[m] + st:i * NK[m] + st + w],
                        start=(i == 0), stop=(i == 3))
                nc.vector.tensor_add(
                    k_sb[m][:, st:st + w], pk[:, 0:w],
                    poskv[m][:, st:st + w])

        def emit_attention(m, c0, cw, post_stage1=None, vjobs=None):
            # pair m, query columns [c0, c0+cw): S^T -> exp -> O, with the
            # O matmuls software-pipelined one s-chunk behind S/exp so the
            # PE never waits on the ACT exp at chunk boundaries.
            nb = cw // 512
            lead = 1 if nb == 2 else 2
            o_a = [opl.tile([65, 512], f32, tag="O", name=f"oa{m}_{c0}_{t}")
                   for t in range(nb)]
            o_b = [opl.tile([65, 512], f32, tag="O", name=f"ob{m}_{c0}_{t}")
                   for t in range(nb)]
            exs = {}

            def s_stage(s):
                if nb == 1:
                    sab = mmp.tile([P, 2 * cw], f32, tag="mm",
                                   name=f"sab{m}_{c0}_{s}")
                    sv = [sab[:, 0:cw], sab[:, cw:2 * cw]]
                else:
                    sa = mmp.tile([P, cw], f32, tag="mm",
                                  name=f"sa{m}_{c0}_{s}")
                    sb_ = mmp.tile([P, cw], f32, tag="mm",
                                   name=f"sb{m}_{c0}_{s}")
                    sv = [sa, sb_]
                for t in range(nb):
                    nc.tensor.matmul(
                        sv[0][:, t * 512:(t + 1) * 512],
                        k_sb[m][0:64, s * P:(s + 1) * P],
                        q_sb[m][0:64, c0 + t * 512:c0 + (t + 1) * 512],
                        start=True, stop=True)
                for t in range(nb):
                    nc.tensor.matmul(
                        sv[1][:, t * 512:(t + 1) * 512],
                        k_sb[m][64:128, s * P:(s + 1) * P],
                        q_sb[m][64:128, c0 + t * 512:c0 + (t + 1) * 512],
                        start=True, stop=True, tile_position=(64, 0))
                ex = epl.tile([P, 2 * cw], bf16, tag="expS",
                              name=f"ex{m}_{c0}_{s}")
                pad_b = small[:, pad_off[m] + s:pad_off[m] + s + 1]
                if nb == 1:
                    # combined sab tile: ONE exp covers both heads (the pad
                    # bias is per-key, identical for both).
                    nc.scalar.activation(
                        out=ex, in_=sab, func=AF.Exp,
                        bias=pad_b, scale=0.125)
                else:
                    nc.scalar.activation(
                        out=ex[:, 0:cw], in_=sv[0], func=AF.Exp,
                        bias=pad_b, scale=0.125)
                    nc.scalar.activation(
                        out=ex[:, cw:2 * cw], in_=sv[1], func=AF.Exp,
                        bias=pad_b, scale=0.125)
                exs[s] = ex

            def o_stage(s):
                ex = exs.pop(s)
                for t in range(nb):
                    nc.tensor.matmul(
                        o_a[t], vhat[m][:, s * 130:s * 130 + 65],
                        ex[:, t * 512:(t + 1) * 512],
                        start=(s == 0), stop=(s == scs[m] - 1))
                for t in range(nb):
                    nc.tensor.matmul(
                        o_b[t], vhat[m][:, s * 130 + 65:s * 130 + 130],
                        ex[:, cw + t * 512:cw + (t + 1) * 512],
                        start=(s == 0), stop=(s == scs[m] - 1))

            for s in range(scs[m]):
                if vjobs and s % 2 == 0 and s // 2 < len(vjobs):
                    vjobs[s // 2]()
                s_stage(s)
                if s == 1 and post_stage1 is not None:
                    post_stage1()
                if s >= lead:
                    o_stage(s - lead)
            for s in range(scs[m] - lead, scs[m]):
                o_stage(s)
            return o_a, o_b

        def emit_normalize(m, c0, o_a, o_b):
            # normalize: h = O[0:64] / l, l = O[64].  Entirely PE-free and
            # per-(head, col-block) so the chains pipeline: DVE copies the
            # denominator row to partition 0, GPSIMD broadcasts it over 64
            # rows, a 64-lane fast approx reciprocal inverts, DVE
            # multiplies.  (Copies run on DVE, never ACT: the scalar
            # engine is saturated with the exp stream.)
            for j, o_ in ((0, o_a), (1, o_b)):
                for t, ot_ in enumerate(o_):
                    l_sb = nrm.tile([1, 512], f32, tag=f"l{j}{t}",
                                    name=f"l{m}_{c0}_{j}{t}")
                    nc.vector.tensor_copy(out=l_sb, in_=ot_[64:65, :])
                    lb = nrm.tile([CH, 512], f32, tag=f"lb{j}{t}",
                                  name=f"lb{m}_{c0}_{j}{t}")
                    nc.gpsimd.partition_broadcast(lb, l_sb)
                    nc.vector.reciprocal_approx_fast(out=lb, in_=lb)
                    nc.vector.tensor_mul(
                        h_pair[m][j * CH:(j + 1) * CH,
                                  c0 + t * 512:c0 + (t + 1) * 512],
                        ot_[0:64, :], lb)

        # static ones columns of vhat (pad keys contribute exp=0, so an
        # all-ones column is correct for every row).  Full-tile memset; the
        # v copies then overwrite every non-ones column.
        for m in range(NMG):
            nc.vector.memset(vhat[m], 1.0)

        emit_qk(0)

        def mk_norm(cm, c0, oab):
            return lambda: emit_normalize(cm, c0, *oab)

        oh0 = emit_attention(0, 0, 512, vjobs=v_jobs(0))
        oh1 = emit_attention(0, 512, 512, post_stage1=mk_norm(0, 0, oh0))
        carry = oh1
        gate(8, nc.vector)          # -> proj weights
        for m in (1, 2):
            emit_qk(m)
            oh0 = emit_attention(
                m, 0, 512, post_stage1=mk_norm(m - 1, 512, carry),
                vjobs=v_jobs(m))
            if m == 1:
                # residual res = x + proj_b, from the bf16 x_q already in
                # SBUF (consumed only by the tail; emitted here in a DVE
                # lull so it never contends with norm chains).
                for i in range(4):
                    nc.vector.tensor_scalar(
                        out=res[:, i * TH:(i + 1) * TH],
                        in0=xq_raw[:, i * TH:(i + 1) * TH],
                        scalar1=pb[i], scalar2=None, op0=OP.add)
            oh1 = emit_attention(
                m, 512, 512, post_stage1=mk_norm(m, 0, oh0))
            carry = oh1
        # group 3 runs its h1 columns FIRST so the final normalize (cols
        # 0:512) overlaps proj(tb1) at the tail.
        emit_qk(3)
        oh1 = emit_attention(
            3, 512, 512, post_stage1=mk_norm(2, 512, carry),
            vjobs=v_jobs(3))
        oh0 = emit_attention(3, 0, 512, post_stage1=mk_norm(3, 512, oh1))
        carry = oh0

        # ---- proj + residual (contraction over 4 stacked pairs), per
        # 512-col block; output batched into one bf16 DMA per tb.
        # proj(tb0) only needs cols 0:512 of every h_pair (normalized by
        # the norm(3, 0) hook inside att(3, h1)), so it runs while the
        # last normalize chain (cols 512:) is still on DVE/GPSIMD.
        def emit_proj(tb):
            for ci in range(4):
                pp = mmp.tile([P, 512], f32, tag="mm", name=f"pp{ci}_{tb}")
                for pm in range(4):
                    nc.tensor.matmul(
                        pp, wp[:, pm * 512 + ci * P:pm * 512 + (ci + 1) * P],
                        h_pair[pm][:, tb * 512:(tb + 1) * 512],
                        start=(pm == 0), stop=(pm == 3))
                nc.vector.tensor_add(
                    ot[tb][:, ci * 512:(ci + 1) * 512], pp,
                    res[:, ci * TH + tb * 512:ci * TH + (tb + 1) * 512])
            nc.sync.dma_start(
                d_out[:, tb * 2048:(tb + 1) * 2048], ot[tb])

        emit_proj(1)
        emit_normalize(NMG - 1, 0, *carry)
        emit_proj(0)

    nc.finalize()
    return nc


def _prepare(inputs):
    """Host-side shard preparation. Returns (nkv, scs, in_maps)."""
    x = np.asarray(inputs["x"], dtype=np.float32)
    pos = np.asarray(inputs["pos"], dtype=np.float32)
    mask = np.asarray(inputs["mask"])
    gn_w = np.asarray(inputs["gn_w"], dtype=np.float32)
    gn_b = np.asarray(inputs["gn_b"], dtype=np.float32)
    qkv_w = np.asarray(inputs["qkv_w"], dtype=np.float32)
    qkv_b = np.asarray(inputs["qkv_b"], dtype=np.float32)
    proj_w = np.asarray(inputs["proj_w"], dtype=np.float32)
    proj_b = np.asarray(inputs["proj_b"], dtype=np.float32)

    # GroupNorm folded to per-channel affine per batch (stats over full T,
    # matching the reference exactly).
    xg = x.reshape(B, NUM_GROUPS, GS, T)
    mu = xg.mean(axis=(2, 3))
    var = xg.var(axis=(2, 3))
    rs = 1.0 / np.sqrt(var + EPS)
    rs_c = np.repeat(rs, GS, axis=1)
    mu_c = np.repeat(mu, GS, axis=1)
    A_all = rs_c * gn_w[None, :]
    B_all = gn_b[None, :] - mu_c * A_all

    # reorder qkv weights: reference splits rows as [h, (q|k|v), 64]; we
    # additionally permute heads into slot order PERM.
    perm = np.asarray(PERM)
    w3 = qkv_w.reshape(H, 3, CH, C)
    b3 = qkv_b.reshape(H, 3, CH)
    wq_r = w3[perm, 0].reshape(C, C)
    wk_r = w3[perm, 1].reshape(C, C)
    wv_r = w3[perm, 2].reshape(C, C)
    bq = b3[perm, 0].reshape(C)
    bk = b3[perm, 1].reshape(C)
    bv = b3[perm, 2].reshape(C)
    wqT = np.ascontiguousarray(wq_r.T)     # [in-ch, slot-out]
    wkT = np.ascontiguousarray(wk_r.T)
    wvT = np.ascontiguousarray(wv_r.T)
    # GroupNorm affine folded into the weights, per batch:
    #   w.T @ (x*A + B) = (A*w).T @ x + (w @ B)
    wqT_b = [wqT * A_all[bb][:, None] for bb in range(B)]
    wkT_b = [wkT * A_all[bb][:, None] for bb in range(B)]
    wvT_b = [wvT * A_all[bb][:, None] for bb in range(B)]
    bqB = [wq_r @ B_all[bb] for bb in range(B)]
    bkB = [wk_r @ B_all[bb] for bb in range(B)]
    bvB = [wv_r @ B_all[bb] for bb in range(B)]
    # proj: input channels permuted to slot order
    perm_idx = (perm[:, None] * CH + np.arange(CH)[None, :]).reshape(-1)
    wpT = np.ascontiguousarray(proj_w.T[perm_idx])  # [slot-in, out-ch]

    # per mask-group key compaction (mask quirk: group m uses mask[m])
    keep = [np.flatnonzero(~mask[m, 0]) for m in range(NMG)]
    scs = tuple(max((len(kp) + P - 1) // P, 1) for kp in keep)
    NK = [s * P for s in scs]
    nkv = max(NK)

    L, stages, TOTAL = _layout(scs)

    def blk4(a):
        # [512, W] -> [128, 4*W] with block i = rows [128i, 128i+128)
        w = a.shape[1]
        return a.reshape(4, P, w).transpose(1, 0, 2).reshape(P, 4 * w)

    def wblk(wT, m):
        # [128, 4*128]: block i = wT[128i:128(i+1), 128m:128(m+1)]
        return np.concatenate(
            [wT[i * P:(i + 1) * P, m * P:(m + 1) * P] for i in range(4)],
            axis=1)

    wp_piece = wpT.reshape(4, P, C).transpose(1, 0, 2).reshape(P, 4 * C)
    ident_piece = np.eye(P, dtype=np.float32)

    # per-batch xkv (shared by the two query-half cores of the batch)
    xkv_b_all = []
    for bb in range(B):
        per_m = []
        for m in range(NMG):
            kp = keep[m]
            buf = np.zeros((C, NK[m]), dtype=np.float32)
            buf[:, :len(kp)] = x[bb][:, kp]
            per_m.append(blk4(buf))
        xkv_b_all.append(per_m)

    # small f32 blob: pb 4 | pad per group
    pb_piece = proj_b.reshape(4, P).T                       # [128, 4]
    SW = 4 + sum(scs)
    sm = np.zeros((P, SW), dtype=np.float32)
    sm[:, 0:4] = pb_piece
    o = 4
    for m in range(NMG):
        padv = np.zeros(NK[m], dtype=np.float32)
        padv[len(keep[m]):] = -1e9
        sm[:, o:o + scs[m]] = padv.reshape(scs[m], P).T
        o += scs[m]

    in_maps = []
    for core in range(8):
        bb, half = core // 2, core % 2
        ts = slice(half * TH, (half + 1) * TH)
        posb = pos[bb * H:(bb + 1) * H]        # [8, 64, 2048] true head order

        pieces = {}
        pieces["xq"] = blk4(x[bb][:, ts])
        pieces["wv"] = np.concatenate(
            [wblk(wvT_b[bb], m) for m in range(NMG)], axis=1)
        pieces["wp"] = wp_piece
        pieces["ident"] = ident_piece
        posq_full = (posb[perm][:, :, ts].reshape(C, TH)
                     + (bq + bqB[bb])[:, None])
        for m in range(NMG):
            pieces[f"wq{m}"] = wblk(wqT_b[bb], m)
            pieces[f"wk{m}"] = wblk(wkT_b[bb], m)
            pieces[f"posq{m}"] = posq_full[m * P:(m + 1) * P]
            pieces[f"xkv{m}"] = xkv_b_all[bb][m]
            kp = keep[m]
            nb = len(kp)
            pkv = np.zeros((P, NK[m]), dtype=np.float32)
            posT = np.zeros((NK[m], P), dtype=np.float32)
            for j, hh in enumerate((m, m + 4)):   # slots 2m, 2m+1
                sl = slice((2 * m + j) * CH, (2 * m + j + 1) * CH)
                pkv[j * CH:(j + 1) * CH, :nb] = (
                    posb[hh][:, kp] + (bk + bkB[bb])[sl][:, None])
                posT[:nb, j * CH:(j + 1) * CH] = (
                    posb[hh][:, kp].T + (bv + bvB[bb])[sl][None, :])
            pieces[f"poskv{m}"] = pkv
            pieces[f"posT{m}"] = posT.reshape(
                scs[m], P, P).transpose(1, 0, 2).reshape(P, scs[m] * P)

        blob = np.empty((P, TOTAL), dtype=BF16)
        for name, (o_, w_) in L.items():
            assert pieces[name].shape == (P, w_), (name, pieces[name].shape, w_)
            blob[:, o_:o_ + w_] = pieces[name].astype(BF16)

        in_maps.append({"blob": blob, "small": sm})
    return nkv, scs, in_maps


def _unpack_out(raw):
    """Device out [128, 2*4*512] bf16 -> [C, TH] float32."""
    o = np.empty((C, TH), dtype=np.float32)
    for tb in range(2):
        for ci in range(4):
            o[ci * P:(ci + 1) * P, tb * 512:(tb + 1) * 512] = \
                raw[:, tb * 2048 + ci * 512:tb * 2048 + (ci + 1) * 512]
    return o


def kernel(**inputs):
    from concourse.bass_utils import run_bass_kernel_spmd

    nkv, scs, in_maps = _prepare(inputs)
    key = scs
    if key not in _graph_cache:
        _graph_cache[key] = _build(nkv, scs)
    nc = _graph_cache[key]

    res = run_bass_kernel_spmd(nc, in_maps, core_ids=list(range(8)))
    results = res.results

    out = np.empty((B, C, T), dtype=np.float32)
    for core in range(8):
        bb, half = core // 2, core % 2
        out[bb][:, half * TH:(half + 1) * TH] = _unpack_out(
            np.asarray(results[core]["out"], dtype=np.float32))
    return out
